# revision 1
# baseline (speedup 1.0000x reference)
"""AIMNet2 interaction module on 8 TRN2 NeuronCores.

Strategy: the reference gathers per-pair features with idx_j and
segment-sums with the SAME idx_j.  Within the segment of atom n every
gathered row equals the per-atom value, so the pairwise work collapses:

  radial_emb[n]  = E[n] * (segsum(gs)[n] @ W_gs.T)
  radial_q[n]    = q[n] * (segsum(gs)[n] @ W_gs.T)
  avf_sum[n,h,d] = sum_g (E @ AGH)[n,g,h] * segsum(gv)[n,d,g]

The only per-pair device work is segment-summing the 64-float payload
[gs | gv].  Pairs are sharded by destination atom (host-side sort), so
each of the 8 cores owns N/8 = 1250 atoms and needs no collectives.
Segment sums are computed with one-hot matmuls on the TensorEngine:
pairs are bucketed into 128-atom windows; a [128pair x 128atom] one-hot
(built on-device by comparing the window-local index against an iota
matrix) is the stationary operand and the payload streams through,
accumulating [128atom x 64] in PSUM.
"""

import sys

if "/opt/trn_rl_repo" not in sys.path:
    sys.path.insert(0, "/opt/trn_rl_repo")

import numpy as np

import concourse.bass as bass
import concourse.bacc as bacc
import concourse.mybir as mybir
import concourse.tile as tile
from concourse.bass_utils import run_bass_kernel_spmd

FP = mybir.dt.float32
N_CORES = 8
N_ATOMS = 10000
F = 256
G = 16
H = 64
HID = 256
OUT_F = F + 2  # 258
APC = N_ATOMS // N_CORES  # 1250 atoms per core
NT = (APC + 127) // 128  # 10 atom tiles (windows) per core
PAD_ATOMS = NT * 128  # 1280

_ALU = mybir.AluOpType
_ACT = mybir.ActivationFunctionType

_cache = {}


def _build(budgets):
    """Build the SPMD graph. budgets[w] = number of 128-pair tiles for window w."""
    nc = bacc.Bacc(None, target_bir_lowering=False, debug=False)
    t_total = sum(budgets)

    pay_d = nc.dram_tensor("payload", [t_total * 128, 65], FP, kind="ExternalInput")
    e_d = nc.dram_tensor("e", [PAD_ATOMS, F], FP, kind="ExternalInput")
    eT_d = nc.dram_tensor("eT", [F, PAD_ATOMS], FP, kind="ExternalInput")
    q_d = nc.dram_tensor("q", [PAD_ATOMS, 1], FP, kind="ExternalInput")
    aghr_d = nc.dram_tensor("aghr", [F, G * H], FP, kind="ExternalInput")
    wgsT_d = nc.dram_tensor("wgsT", [G, F], FP, kind="ExternalInput")
    w1T_d = nc.dram_tensor("w1T", [640, HID], FP, kind="ExternalInput")
    b1_d = nc.dram_tensor("b1", [HID, 1], FP, kind="ExternalInput")
    w2T_d = nc.dram_tensor("w2T", [HID, HID], FP, kind="ExternalInput")
    b2_d = nc.dram_tensor("b2", [HID, 1], FP, kind="ExternalInput")
    w3T_d = nc.dram_tensor("w3T", [HID, OUT_F], FP, kind="ExternalInput")
    b3a_d = nc.dram_tensor("b3a", [F, 1], FP, kind="ExternalInput")
    b3b_d = nc.dram_tensor("b3b", [2, 1], FP, kind="ExternalInput")
    iota_d = nc.dram_tensor("iota", [128, 128], FP, kind="ExternalInput")
    ident_d = nc.dram_tensor("ident", [128, 128], FP, kind="ExternalInput")
    out_d = nc.dram_tensor("out", [OUT_F, PAD_ATOMS], FP, kind="ExternalOutput")

    with tile.TileContext(nc) as tc:
        with (
            tc.tile_pool(name="const", bufs=1) as cpool,
            tc.tile_pool(name="pay", bufs=NT) as paypool,
            tc.tile_pool(name="work", bufs=3) as wpool,
            tc.tile_pool(name="oh", bufs=4) as ohpool,
            tc.tile_pool(name="ps_seg", bufs=2, space="PSUM") as ps_seg,
            tc.tile_pool(name="ps_big", bufs=2, space="PSUM") as ps_big,
            tc.tile_pool(name="ps_mm", bufs=4, space="PSUM") as ps_mm,
        ):
            e_sb = cpool.tile([128, NT, F], FP)
            nc.sync.dma_start(e_sb[:], e_d[:].rearrange("(t p) f -> p t f", p=128))
            eT_sb = cpool.tile([128, 2, PAD_ATOMS], FP)
            nc.sync.dma_start(eT_sb[:], eT_d[:].rearrange("(c p) n -> p c n", p=128))
            q_sb = cpool.tile([128, NT, 1], FP)
            nc.sync.dma_start(q_sb[:], q_d[:].rearrange("(t p) o -> p t o", p=128))
            aghr_sb = cpool.tile([128, 2, G * H], FP)
            nc.sync.dma_start(aghr_sb[:], aghr_d[:].rearrange("(c p) n -> p c n", p=128))
            wgsT_sb = cpool.tile([G, F], FP)
            nc.sync.dma_start(wgsT_sb[:], wgsT_d[:])
            w1T_sb = cpool.tile([128, 5, HID], FP)
            nc.sync.dma_start(w1T_sb[:], w1T_d[:].rearrange("(c p) n -> p c n", p=128))
            b1_sb = cpool.tile([128, 2, 1], FP)
            nc.sync.dma_start(b1_sb[:], b1_d[:].rearrange("(c p) o -> p c o", p=128))
            w2T_sb = cpool.tile([128, 2, HID], FP)
            nc.sync.dma_start(w2T_sb[:], w2T_d[:].rearrange("(c p) n -> p c n", p=128))
            b2_sb = cpool.tile([128, 2, 1], FP)
            nc.sync.dma_start(b2_sb[:], b2_d[:].rearrange("(c p) o -> p c o", p=128))
            w3T_sb = cpool.tile([128, 2, OUT_F], FP)
            nc.sync.dma_start(w3T_sb[:], w3T_d[:].rearrange("(c p) n -> p c n", p=128))
            b3a_sb = cpool.tile([128, 2, 1], FP)
            nc.sync.dma_start(b3a_sb[:], b3a_d[:].rearrange("(c p) o -> p c o", p=128))
            b3b_sb = cpool.tile([2, 1], FP)
            nc.sync.dma_start(b3b_sb[:], b3b_d[:])
            iota_sb = cpool.tile([128, 128], FP)
            nc.sync.dma_start(iota_sb[:], iota_d[:])
            ident_sb = cpool.tile([128, 128], FP)
            nc.sync.dma_start(ident_sb[:], ident_d[:])

            off = 0
            for t in range(NT):
                B = budgets[t]
                pay_sb = paypool.tile([128, B, 65], FP, tag="pay")
                nc.sync.dma_start(
                    pay_sb[:],
                    pay_d[off * 128 : (off + B) * 128, :].rearrange(
                        "(b p) c -> p b c", p=128
                    ),
                )

                # --- stage A: segment-sum [gs|gv] over this 128-atom window
                sums_ps = ps_seg.tile([128, 64], FP, tag="seg")
                for b in range(B):
                    oh = ohpool.tile([128, 128], FP, tag="oh")
                    nc.gpsimd.tensor_scalar(
                        oh[:], iota_sb[:], pay_sb[:, b, 64:65], None, _ALU.is_equal
                    )
                    nc.tensor.matmul(
                        sums_ps[:],
                        oh[:],
                        pay_sb[:, b, 0:64],
                        start=(b == 0),
                        stop=(b == B - 1),
                    )
                sums_sb = wpool.tile([128, 64], FP, tag="sums")
                nc.any.tensor_copy(sums_sb[:], sums_ps[:])

                # --- stage B: M = E @ AGH for this atom tile, [128, 1024]
                m_sb = wpool.tile([128, G * H], FP, tag="m")
                for nh in range(2):
                    mp = ps_big.tile([128, 512], FP, tag="mps")
                    for kc in range(2):
                        nc.tensor.matmul(
                            mp[:],
                            eT_sb[:, kc, t * 128 : (t + 1) * 128],
                            aghr_sb[:, kc, nh * 512 : (nh + 1) * 512],
                            start=(kc == 0),
                            stop=(kc == 1),
                        )
                    nc.any.tensor_copy(m_sb[:, nh * 512 : (nh + 1) * 512], mp[:])

                # --- stage C: feature-major MLP input chunks (xT)
                # xT chunks: 0,1 = radial_emb.T; 2,3 = radial_q.T; 4 = vector_emb.T
                qgs_sb = wpool.tile([128, G], FP, tag="qgs")
                nc.vector.tensor_scalar(
                    qgs_sb[:], sums_sb[:, 0:G], q_sb[:, t, :], None, _ALU.mult
                )
                gsT_ps = ps_mm.tile([G, 128], FP, tag="psmm")
                nc.tensor.transpose(gsT_ps[:], sums_sb[:, 0:G], ident_sb[:])
                gsT_sb = wpool.tile([G, 128], FP, tag="gsT")
                nc.any.tensor_copy(gsT_sb[:], gsT_ps[:])
                qgsT_ps = ps_mm.tile([G, 128], FP, tag="psmm")
                nc.tensor.transpose(qgsT_ps[:], qgs_sb[:], ident_sb[:])
                qgsT_sb = wpool.tile([G, 128], FP, tag="qgsT")
                nc.any.tensor_copy(qgsT_sb[:], qgsT_ps[:])

                xT_sb = wpool.tile([128, 5, 128], FP, tag="xT")
                for c in range(2):
                    mg = ps_mm.tile([128, 128], FP, tag="psmm")
                    nc.tensor.matmul(
                        mg[:], wgsT_sb[:, c * 128 : (c + 1) * 128], gsT_sb[:],
                        start=True, stop=True,
                    )
                    nc.vector.tensor_tensor(
                        xT_sb[:, c, :], mg[:],
                        eT_sb[:, c, t * 128 : (t + 1) * 128], _ALU.mult,
                    )
                for c in range(2):
                    mg = ps_mm.tile([128, 128], FP, tag="psmm")
                    nc.tensor.matmul(
                        mg[:], wgsT_sb[:, c * 128 : (c + 1) * 128], qgsT_sb[:],
                        start=True, stop=True,
                    )
                    nc.any.tensor_copy(xT_sb[:, 2 + c, :], mg[:])

                # --- stage D: avf contraction + vector norm -> xT chunk 4
                avf_sb = wpool.tile([128, 3 * H], FP, tag="avf")
                for d in range(3):
                    av = avf_sb[:, d * H : (d + 1) * H]
                    for g in range(G):
                        msl = m_sb[:, g * H : (g + 1) * H]
                        sc = sums_sb[:, G + d * G + g : G + d * G + g + 1]
                        if g == 0:
                            nc.vector.tensor_scalar(av, msl, sc, None, _ALU.mult)
                        else:
                            nc.vector.scalar_tensor_tensor(
                                av, msl, sc, av, _ALU.mult, _ALU.add
                            )
                sq_sb = wpool.tile([128, 3 * H], FP, tag="sq")
                nc.scalar.activation(sq_sb[:], avf_sb[:], _ACT.Square)
                s01_sb = wpool.tile([128, H], FP, tag="s01")
                nc.vector.tensor_tensor(
                    s01_sb[:], sq_sb[:, 0:H], sq_sb[:, H : 2 * H], _ALU.add
                )
                s2_sb = wpool.tile([128, H], FP, tag="s2")
                nc.vector.tensor_tensor(
                    s2_sb[:], s01_sb[:], sq_sb[:, 2 * H : 3 * H], _ALU.add
                )
                vr_sb = wpool.tile([128, H], FP, tag="vr")
                nc.scalar.activation(vr_sb[:], s2_sb[:], _ACT.Sqrt)
                vT_ps = ps_mm.tile([H, 128], FP, tag="psmm")
                nc.tensor.transpose(vT_ps[:], vr_sb[:], ident_sb[:])
                nc.any.tensor_copy(xT_sb[0:H, 4, :], vT_ps[:])

                # --- stage E: MLP (feature-major, no transposes between layers)
                h1_sb = wpool.tile([128, 2, 128], FP, tag="h1")
                for m in range(2):
                    ps = ps_mm.tile([128, 128], FP, tag="psmm")
                    for kc in range(5):
                        kk = 128 if kc < 4 else H
                        nc.tensor.matmul(
                            ps[:],
                            w1T_sb[0:kk, kc, m * 128 : (m + 1) * 128],
                            xT_sb[0:kk, kc, :],
                            start=(kc == 0),
                            stop=(kc == 4),
                        )
                    nc.scalar.activation(
                        h1_sb[:, m, :], ps[:], _ACT.Gelu, bias=b1_sb[:, m, :]
                    )
                h2_sb = wpool.tile([128, 2, 128], FP, tag="h2")
                for m in range(2):
                    ps = ps_mm.tile([128, 128], FP, tag="psmm")
                    for kc in range(2):
                        nc.tensor.matmul(
                            ps[:],
                            w2T_sb[:, kc, m * 128 : (m + 1) * 128],
                            h1_sb[:, kc, :],
                            start=(kc == 0),
                            stop=(kc == 1),
                        )
                    nc.scalar.activation(
                        h2_sb[:, m, :], ps[:], _ACT.Gelu, bias=b2_sb[:, m, :]
                    )
                for m in range(2):
                    ps = ps_mm.tile([128, 128], FP, tag="psmm")
                    for kc in range(2):
                        nc.tensor.matmul(
                            ps[:],
                            w3T_sb[:, kc, m * 128 : (m + 1) * 128],
                            h2_sb[:, kc, :],
                            start=(kc == 0),
                            stop=(kc == 1),
                        )
                    o_sb = wpool.tile([128, 128], FP, tag="osb")
                    nc.vector.tensor_scalar(
                        o_sb[:], ps[:], b3a_sb[:, m, :], None, _ALU.add
                    )
                    nc.sync.dma_start(
                        out_d[m * 128 : (m + 1) * 128, t * 128 : (t + 1) * 128],
                        o_sb[:],
                    )
                ps2 = ps_mm.tile([2, 128], FP, tag="psmm")
                for kc in range(2):
                    nc.tensor.matmul(
                        ps2[:],
                        w3T_sb[:, kc, F : F + 2],
                        h2_sb[:, kc, :],
                        start=(kc == 0),
                        stop=(kc == 1),
                    )
                o2_sb = wpool.tile([2, 128], FP, tag="o2")
                nc.vector.tensor_scalar(o2_sb[:], ps2[:], b3b_sb[:], None, _ALU.add)
                nc.sync.dma_start(
                    out_d[F : F + 2, t * 128 : (t + 1) * 128], o2_sb[:]
                )
                off += B

    nc.compile()
    return nc


def _prep(atomic_embedding, partial_charges, pair_indices, gs, gv, agh,
          W_gs, W1, b1, W2, b2, W3, b3):
    E = np.ascontiguousarray(np.asarray(atomic_embedding, dtype=np.float32))
    q = np.asarray(partial_charges, dtype=np.float32).reshape(N_ATOMS, 1)
    idx = np.asarray(pair_indices)[1].astype(np.int64)
    n_pairs = idx.shape[0]
    gs = np.asarray(gs, dtype=np.float32)
    gv = np.asarray(gv, dtype=np.float32).reshape(n_pairs, 3 * G)

    order = np.argsort(idx, kind="stable")
    idx_s = idx[order]
    pay_all = np.empty((n_pairs, 65), dtype=np.float32)
    pay_all[:, 0:G] = gs[order]
    pay_all[:, G:64] = gv[order]

    # window boundaries: core k, window w covers atoms [k*APC + w*128, ...)
    bounds = np.zeros((N_CORES, NT + 1), dtype=np.int64)
    counts = np.zeros((N_CORES, NT), dtype=np.int64)
    for k in range(N_CORES):
        for w in range(NT):
            lo = k * APC + w * 128
            hi = min(k * APC + (w + 1) * 128, (k + 1) * APC)
            bounds[k, w] = np.searchsorted(idx_s, lo)
            if w == NT - 1:
                bounds[k, NT] = np.searchsorted(idx_s, hi)
        counts[k] = np.diff(bounds[k])
    budgets = tuple(
        int(max(1, -(-int(counts[:, w].max()) // 128))) for w in range(NT)
    )
    t_total = sum(budgets)

    # shared params
    aghr = np.ascontiguousarray(
        np.asarray(agh, dtype=np.float32).reshape(F, G * H)
    )
    wgsT = np.ascontiguousarray(np.asarray(W_gs, dtype=np.float32).T)
    W1 = np.asarray(W1, dtype=np.float32)
    # permute MLP input features: [radial_emb, radial_q, vector_emb], drop vector_q
    W1p = np.concatenate([W1[:, 0:F], W1[:, F + H : 2 * F + H], W1[:, F : F + H]], axis=1)
    w1T = np.zeros((640, HID), dtype=np.float32)
    w1T[0 : 2 * F + H] = W1p.T
    w2T = np.ascontiguousarray(np.asarray(W2, dtype=np.float32).T)
    w3T = np.ascontiguousarray(np.asarray(W3, dtype=np.float32).T)
    b1v = np.asarray(b1, dtype=np.float32).reshape(HID, 1)
    b2v = np.asarray(b2, dtype=np.float32).reshape(HID, 1)
    b3v = np.asarray(b3, dtype=np.float32).reshape(OUT_F, 1)
    iota = np.tile(np.arange(128, dtype=np.float32), (128, 1))
    ident = np.eye(128, dtype=np.float32)

    in_maps = []
    for k in range(N_CORES):
        pay = np.zeros((t_total * 128, 65), dtype=np.float32)
        off = 0
        for w in range(NT):
            lo_p, hi_p = bounds[k, w], bounds[k, w + 1]
            cnt = hi_p - lo_p
            pay[off * 128 : off * 128 + cnt, 0:64] = pay_all[lo_p:hi_p, 0:64]
            pay[off * 128 : off * 128 + cnt, 64] = (
                idx_s[lo_p:hi_p] - (k * APC + w * 128)
            ).astype(np.float32)
            off += budgets[w]
        e_k = np.zeros((PAD_ATOMS, F), dtype=np.float32)
        e_k[0:APC] = E[k * APC : (k + 1) * APC]
        q_k = np.zeros((PAD_ATOMS, 1), dtype=np.float32)
        q_k[0:APC] = q[k * APC : (k + 1) * APC]
        in_maps.append(
            {
                "payload": pay,
                "e": e_k,
                "eT": np.ascontiguousarray(e_k.T),
                "q": q_k,
                "aghr": aghr,
                "wgsT": wgsT,
                "w1T": w1T,
                "b1": b1v,
                "w2T": w2T,
                "b2": b2v,
                "w3T": w3T,
                "b3a": b3v[0:F],
                "b3b": b3v[F : F + 2],
                "iota": iota,
                "ident": ident,
            }
        )
    return budgets, in_maps


def _run(inputs, trace=False):
    budgets, in_maps = _prep(**inputs)
    if budgets not in _cache:
        _cache[budgets] = _build(list(budgets))
    nc = _cache[budgets]
    res = run_bass_kernel_spmd(
        nc, in_maps, core_ids=list(range(N_CORES)), trace=trace
    )
    outs = [res.results[k]["out"] for k in range(N_CORES)]
    full = np.concatenate([o[:, :APC] for o in outs], axis=1).T
    full = np.ascontiguousarray(full, dtype=np.float32)
    delta_q = full[:, 0:1]
    f_out = full[:, 1:2]
    delta_a = full[:, 2:]
    return (delta_a, delta_q, f_out), res


def kernel(**inputs):
    out, _ = _run(inputs, trace=False)
    return out


# revision 4
# speedup vs baseline: 2.4681x; 2.4681x over previous
"""AIMNet2 interaction module on 8 TRN2 NeuronCores.

Strategy: the reference gathers per-pair features with idx_j and
segment-sums with the SAME idx_j.  Within the segment of atom n every
gathered row equals the per-atom value, so the pairwise work collapses:

  radial_emb[n]  = E[n] * (segsum(gs)[n] @ W_gs.T)
  radial_q[n]    = q[n] * (segsum(gs)[n] @ W_gs.T)
  avf_sum[n,h,d] = sum_g (E @ AGH)[n,g,h] * segsum(gv)[n,d,g]

The only per-pair device work is segment-summing the 64-float payload
[gs | gv].  Pairs are sharded by destination atom (host-side sort), so
each of the 8 cores owns N/8 = 1250 atoms and needs no collectives.
Segment sums are computed with one-hot matmuls on the TensorEngine:
pairs are bucketed into 128-atom windows; a [128pair x 128atom] one-hot
(built on-device by comparing the window-local index against an iota
matrix) is the stationary operand and the payload streams through,
accumulating [128atom x 64] in PSUM.

All TensorEngine-facing tensors are bf16 (fp32 matmul runs ~4x slower
on the PE and disables fast weight load); accumulation stays fp32 in
PSUM, and the MLP output is produced in fp32.
"""

import sys

if "/opt/trn_rl_repo" not in sys.path:
    sys.path.insert(0, "/opt/trn_rl_repo")

import numpy as np

import concourse.bass as bass
import concourse.bacc as bacc
import concourse.mybir as mybir
import concourse.tile as tile
from concourse.bass_utils import run_bass_kernel_spmd

FP = mybir.dt.float32
BF = mybir.dt.bfloat16
NP_BF = mybir.dt.np(BF)
N_CORES = 8
N_ATOMS = 10000
F = 256
G = 16
H = 64
HID = 256
OUT_F = F + 2  # 258
APC = N_ATOMS // N_CORES  # 1250 atoms per core
NT = (APC + 127) // 128  # 10 atom tiles (windows) per core
PAD_ATOMS = NT * 128  # 1280

_ALU = mybir.AluOpType
_ACT = mybir.ActivationFunctionType

_cache = {}


def _build(budgets):
    """Build the SPMD graph. budgets[w] = number of 128-pair tiles for window w."""
    nc = bacc.Bacc(None, target_bir_lowering=False, debug=False)
    t_total = sum(budgets)

    pay_d = nc.dram_tensor("payload", [t_total * 128, 64], BF, kind="ExternalInput")
    pidx_d = nc.dram_tensor("pidx", [t_total * 128, 1], FP, kind="ExternalInput")
    eT_d = nc.dram_tensor("eT", [F, PAD_ATOMS], BF, kind="ExternalInput")
    q_d = nc.dram_tensor("q", [PAD_ATOMS, 1], FP, kind="ExternalInput")
    aghr_d = nc.dram_tensor("aghr", [F, G * H], BF, kind="ExternalInput")
    wgsT_d = nc.dram_tensor("wgsT", [G, F], BF, kind="ExternalInput")
    w1T_d = nc.dram_tensor("w1T", [640, HID], BF, kind="ExternalInput")
    b1_d = nc.dram_tensor("b1", [HID, 1], FP, kind="ExternalInput")
    w2T_d = nc.dram_tensor("w2T", [HID, HID], BF, kind="ExternalInput")
    b2_d = nc.dram_tensor("b2", [HID, 1], FP, kind="ExternalInput")
    w3T_d = nc.dram_tensor("w3T", [HID, OUT_F], BF, kind="ExternalInput")
    b3a_d = nc.dram_tensor("b3a", [F, 1], FP, kind="ExternalInput")
    b3b_d = nc.dram_tensor("b3b", [2, 1], FP, kind="ExternalInput")
    iota_d = nc.dram_tensor("iota", [128, 128], BF, kind="ExternalInput")
    ident_d = nc.dram_tensor("ident", [128, 128], FP, kind="ExternalInput")
    out_d = nc.dram_tensor("out", [OUT_F, PAD_ATOMS], FP, kind="ExternalOutput")

    with tile.TileContext(nc) as tc:
        with (
            tc.tile_pool(name="const", bufs=1) as cpool,
            tc.tile_pool(name="pay", bufs=NT) as paypool,
            tc.tile_pool(name="work", bufs=3) as wpool,
            tc.tile_pool(name="oh", bufs=4) as ohpool,
            tc.tile_pool(name="ps_seg", bufs=2, space="PSUM") as ps_seg,
            tc.tile_pool(name="ps_big", bufs=2, space="PSUM") as ps_big,
            tc.tile_pool(name="ps_mm", bufs=4, space="PSUM") as ps_mm,
        ):
            eT_sb = cpool.tile([128, 2, PAD_ATOMS], BF)
            nc.sync.dma_start(eT_sb[:], eT_d[:].rearrange("(c p) n -> p c n", p=128))
            q_sb = cpool.tile([128, NT, 1], FP)
            nc.sync.dma_start(q_sb[:], q_d[:].rearrange("(t p) o -> p t o", p=128))
            aghr_sb = cpool.tile([128, 2, G * H], BF)
            nc.sync.dma_start(aghr_sb[:], aghr_d[:].rearrange("(c p) n -> p c n", p=128))
            wgsT_sb = cpool.tile([G, F], BF)
            nc.sync.dma_start(wgsT_sb[:], wgsT_d[:])
            w1T_sb = cpool.tile([128, 5, HID], BF)
            nc.sync.dma_start(w1T_sb[:], w1T_d[:].rearrange("(c p) n -> p c n", p=128))
            b1_sb = cpool.tile([128, 2, 1], FP)
            nc.sync.dma_start(b1_sb[:], b1_d[:].rearrange("(c p) o -> p c o", p=128))
            w2T_sb = cpool.tile([128, 2, HID], BF)
            nc.sync.dma_start(w2T_sb[:], w2T_d[:].rearrange("(c p) n -> p c n", p=128))
            b2_sb = cpool.tile([128, 2, 1], FP)
            nc.sync.dma_start(b2_sb[:], b2_d[:].rearrange("(c p) o -> p c o", p=128))
            w3T_sb = cpool.tile([128, 2, OUT_F], BF)
            nc.sync.dma_start(w3T_sb[:], w3T_d[:].rearrange("(c p) n -> p c n", p=128))
            b3a_sb = cpool.tile([128, 2, 1], FP)
            nc.sync.dma_start(b3a_sb[:], b3a_d[:].rearrange("(c p) o -> p c o", p=128))
            b3b_sb = cpool.tile([2, 1], FP)
            nc.sync.dma_start(b3b_sb[:], b3b_d[:])
            iota_sb = cpool.tile([128, 128], BF)
            nc.sync.dma_start(iota_sb[:], iota_d[:])
            ident_sb = cpool.tile([128, 128], FP)
            nc.sync.dma_start(ident_sb[:], ident_d[:])

            off = 0
            for t in range(NT):
                B = budgets[t]
                pay_sb = paypool.tile([128, B, 64], BF, tag="pay")
                nc.sync.dma_start(
                    pay_sb[:],
                    pay_d[off * 128 : (off + B) * 128, :].rearrange(
                        "(b p) c -> p b c", p=128
                    ),
                )
                pidx_sb = paypool.tile([128, B, 1], FP, tag="pidx")
                nc.sync.dma_start(
                    pidx_sb[:],
                    pidx_d[off * 128 : (off + B) * 128, :].rearrange(
                        "(b p) o -> p b o", p=128
                    ),
                )

                # --- stage A: segment-sum [gs|gv] over this 128-atom window
                sums_ps = ps_seg.tile([128, 64], FP, tag="seg")
                for b in range(B):
                    oh = ohpool.tile([128, 128], BF, tag="oh")
                    nc.vector.tensor_scalar(
                        oh[:], iota_sb[:], pidx_sb[:, b, :], None, _ALU.is_equal
                    )
                    nc.tensor.matmul(
                        sums_ps[:],
                        oh[:],
                        pay_sb[:, b, :],
                        start=(b == 0),
                        stop=(b == B - 1),
                    )
                sums_sb = wpool.tile([128, 64], FP, tag="sums")
                nc.any.tensor_copy(sums_sb[:], sums_ps[:])

                # --- stage B: M = E @ AGH for this atom tile, [128, 1024] bf16
                m_sb = wpool.tile([128, G * H], BF, tag="m")
                for nh in range(2):
                    mp = ps_big.tile([128, 512], FP, tag="mps")
                    for kc in range(2):
                        nc.tensor.matmul(
                            mp[:],
                            eT_sb[:, kc, t * 128 : (t + 1) * 128],
                            aghr_sb[:, kc, nh * 512 : (nh + 1) * 512],
                            start=(kc == 0),
                            stop=(kc == 1),
                        )
                    nc.any.tensor_copy(m_sb[:, nh * 512 : (nh + 1) * 512], mp[:])

                # --- stage C: feature-major MLP input chunks (xT, bf16)
                # xT chunks: 0,1 = radial_emb.T; 2,3 = radial_q.T; 4 = vector_emb.T
                qgs_sb = wpool.tile([128, G], FP, tag="qgs")
                nc.vector.tensor_scalar(
                    qgs_sb[:], sums_sb[:, 0:G], q_sb[:, t, :], None, _ALU.mult
                )
                gsT_ps = ps_mm.tile([G, 128], FP, tag="psmm")
                nc.tensor.transpose(gsT_ps[:], sums_sb[:, 0:G], ident_sb[:])
                gsT_sb = wpool.tile([G, 128], BF, tag="gsT")
                nc.any.tensor_copy(gsT_sb[:], gsT_ps[:])
                qgsT_ps = ps_mm.tile([G, 128], FP, tag="psmm")
                nc.tensor.transpose(qgsT_ps[:], qgs_sb[:], ident_sb[:])
                qgsT_sb = wpool.tile([G, 128], BF, tag="qgsT")
                nc.any.tensor_copy(qgsT_sb[:], qgsT_ps[:])

                xT_sb = wpool.tile([128, 5, 128], BF, tag="xT")
                for c in range(2):
                    mg = ps_mm.tile([128, 128], FP, tag="psmm")
                    nc.tensor.matmul(
                        mg[:], wgsT_sb[:, c * 128 : (c + 1) * 128], gsT_sb[:],
                        start=True, stop=True,
                    )
                    nc.vector.tensor_tensor(
                        xT_sb[:, c, :], mg[:],
                        eT_sb[:, c, t * 128 : (t + 1) * 128], _ALU.mult,
                    )
                for c in range(2):
                    mg = ps_mm.tile([128, 128], FP, tag="psmm")
                    nc.tensor.matmul(
                        mg[:], wgsT_sb[:, c * 128 : (c + 1) * 128], qgsT_sb[:],
                        start=True, stop=True,
                    )
                    nc.any.tensor_copy(xT_sb[:, 2 + c, :], mg[:])

                # --- stage D: avf contraction + vector norm -> xT chunk 4
                avf_sb = wpool.tile([128, 3 * H], BF, tag="avf")
                for d in range(3):
                    av = avf_sb[:, d * H : (d + 1) * H]
                    for g in range(G):
                        msl = m_sb[:, g * H : (g + 1) * H]
                        sc = sums_sb[:, G + d * G + g : G + d * G + g + 1]
                        if g == 0:
                            nc.vector.tensor_scalar(av, msl, sc, None, _ALU.mult)
                        else:
                            nc.vector.scalar_tensor_tensor(
                                av, msl, sc, av, _ALU.mult, _ALU.add
                            )
                sq_sb = wpool.tile([128, 3 * H], FP, tag="sq")
                nc.vector.tensor_tensor(sq_sb[:], avf_sb[:], avf_sb[:], _ALU.mult)
                s01_sb = wpool.tile([128, H], FP, tag="s01")
                nc.vector.tensor_tensor(
                    s01_sb[:], sq_sb[:, 0:H], sq_sb[:, H : 2 * H], _ALU.add
                )
                s2_sb = wpool.tile([128, H], FP, tag="s2")
                nc.vector.tensor_tensor(
                    s2_sb[:], s01_sb[:], sq_sb[:, 2 * H : 3 * H], _ALU.add
                )
                vr_sb = wpool.tile([128, H], FP, tag="vr")
                nc.scalar.activation(vr_sb[:], s2_sb[:], _ACT.Sqrt)
                vT_ps = ps_mm.tile([H, 128], FP, tag="psmm")
                nc.tensor.transpose(vT_ps[:], vr_sb[:], ident_sb[:])
                nc.any.tensor_copy(xT_sb[0:H, 4, :], vT_ps[:])

                # --- stage E: MLP (feature-major, no transposes between layers)
                h1_sb = wpool.tile([128, 2, 128], BF, tag="h1")
                for m in range(2):
                    ps = ps_mm.tile([128, 128], FP, tag="psmm")
                    for kc in range(5):
                        kk = 128 if kc < 4 else H
                        nc.tensor.matmul(
                            ps[:],
                            w1T_sb[0:kk, kc, m * 128 : (m + 1) * 128],
                            xT_sb[0:kk, kc, :],
                            start=(kc == 0),
                            stop=(kc == 4),
                        )
                    nc.scalar.activation(
                        h1_sb[:, m, :], ps[:], _ACT.Gelu, bias=b1_sb[:, m, :]
                    )
                h2_sb = wpool.tile([128, 2, 128], BF, tag="h2")
                for m in range(2):
                    ps = ps_mm.tile([128, 128], FP, tag="psmm")
                    for kc in range(2):
                        nc.tensor.matmul(
                            ps[:],
                            w2T_sb[:, kc, m * 128 : (m + 1) * 128],
                            h1_sb[:, kc, :],
                            start=(kc == 0),
                            stop=(kc == 1),
                        )
                    nc.scalar.activation(
                        h2_sb[:, m, :], ps[:], _ACT.Gelu, bias=b2_sb[:, m, :]
                    )
                for m in range(2):
                    ps = ps_mm.tile([128, 128], FP, tag="psmm")
                    for kc in range(2):
                        nc.tensor.matmul(
                            ps[:],
                            w3T_sb[:, kc, m * 128 : (m + 1) * 128],
                            h2_sb[:, kc, :],
                            start=(kc == 0),
                            stop=(kc == 1),
                        )
                    o_sb = wpool.tile([128, 128], FP, tag="osb")
                    nc.vector.tensor_scalar(
                        o_sb[:], ps[:], b3a_sb[:, m, :], None, _ALU.add
                    )
                    nc.sync.dma_start(
                        out_d[m * 128 : (m + 1) * 128, t * 128 : (t + 1) * 128],
                        o_sb[:],
                    )
                ps2 = ps_mm.tile([2, 128], FP, tag="psmm")
                for kc in range(2):
                    nc.tensor.matmul(
                        ps2[:],
                        w3T_sb[:, kc, F : F + 2],
                        h2_sb[:, kc, :],
                        start=(kc == 0),
                        stop=(kc == 1),
                    )
                o2_sb = wpool.tile([2, 128], FP, tag="o2")
                nc.vector.tensor_scalar(o2_sb[:], ps2[:], b3b_sb[:], None, _ALU.add)
                nc.sync.dma_start(
                    out_d[F : F + 2, t * 128 : (t + 1) * 128], o2_sb[:]
                )
                off += B

    nc.compile()
    return nc


def _prep(atomic_embedding, partial_charges, pair_indices, gs, gv, agh,
          W_gs, W1, b1, W2, b2, W3, b3):
    E = np.ascontiguousarray(np.asarray(atomic_embedding, dtype=np.float32))
    q = np.asarray(partial_charges, dtype=np.float32).reshape(N_ATOMS, 1)
    idx = np.asarray(pair_indices)[1].astype(np.int64)
    n_pairs = idx.shape[0]
    gs = np.asarray(gs, dtype=np.float32)
    gv = np.asarray(gv, dtype=np.float32).reshape(n_pairs, 3 * G)

    order = np.argsort(idx, kind="stable")
    idx_s = idx[order]
    pay_all = np.empty((n_pairs, 64), dtype=np.float32)
    pay_all[:, 0:G] = gs[order]
    pay_all[:, G:64] = gv[order]

    # window boundaries: core k, window w covers atoms [k*APC + w*128, ...)
    bounds = np.zeros((N_CORES, NT + 1), dtype=np.int64)
    counts = np.zeros((N_CORES, NT), dtype=np.int64)
    for k in range(N_CORES):
        for w in range(NT):
            lo = k * APC + w * 128
            hi = min(k * APC + (w + 1) * 128, (k + 1) * APC)
            bounds[k, w] = np.searchsorted(idx_s, lo)
            if w == NT - 1:
                bounds[k, NT] = np.searchsorted(idx_s, hi)
        counts[k] = np.diff(bounds[k])
    budgets = tuple(
        int(max(1, -(-int(counts[:, w].max()) // 128))) for w in range(NT)
    )
    t_total = sum(budgets)

    # shared params
    aghr = np.asarray(agh, dtype=np.float32).reshape(F, G * H).astype(NP_BF)
    wgsT = np.asarray(W_gs, dtype=np.float32).T.astype(NP_BF)
    W1 = np.asarray(W1, dtype=np.float32)
    # permute MLP input features: [radial_emb, radial_q, vector_emb], drop vector_q
    W1p = np.concatenate([W1[:, 0:F], W1[:, F + H : 2 * F + H], W1[:, F : F + H]], axis=1)
    w1T = np.zeros((640, HID), dtype=NP_BF)
    w1T[0 : 2 * F + H] = W1p.T.astype(NP_BF)
    w2T = np.asarray(W2, dtype=np.float32).T.astype(NP_BF)
    w3T = np.asarray(W3, dtype=np.float32).T.astype(NP_BF)
    b1v = np.asarray(b1, dtype=np.float32).reshape(HID, 1)
    b2v = np.asarray(b2, dtype=np.float32).reshape(HID, 1)
    b3v = np.asarray(b3, dtype=np.float32).reshape(OUT_F, 1)
    iota = np.tile(np.arange(128, dtype=np.float32), (128, 1)).astype(NP_BF)
    ident = np.eye(128, dtype=np.float32)

    in_maps = []
    for k in range(N_CORES):
        pay = np.zeros((t_total * 128, 64), dtype=np.float32)
        pidx = np.zeros((t_total * 128, 1), dtype=np.float32)
        off = 0
        for w in range(NT):
            lo_p, hi_p = bounds[k, w], bounds[k, w + 1]
            cnt = hi_p - lo_p
            pay[off * 128 : off * 128 + cnt] = pay_all[lo_p:hi_p]
            pidx[off * 128 : off * 128 + cnt, 0] = (
                idx_s[lo_p:hi_p] - (k * APC + w * 128)
            ).astype(np.float32)
            off += budgets[w]
        e_k = np.zeros((PAD_ATOMS, F), dtype=np.float32)
        e_k[0:APC] = E[k * APC : (k + 1) * APC]
        q_k = np.zeros((PAD_ATOMS, 1), dtype=np.float32)
        q_k[0:APC] = q[k * APC : (k + 1) * APC]
        in_maps.append(
            {
                "payload": pay.astype(NP_BF),
                "pidx": pidx,
                "eT": np.ascontiguousarray(e_k.T).astype(NP_BF),
                "q": q_k,
                "aghr": aghr,
                "wgsT": wgsT,
                "w1T": w1T,
                "b1": b1v,
                "w2T": w2T,
                "b2": b2v,
                "w3T": w3T,
                "b3a": b3v[0:F],
                "b3b": b3v[F : F + 2],
                "iota": iota,
                "ident": ident,
            }
        )
    return budgets, in_maps


def _run(inputs, trace=False):
    budgets, in_maps = _prep(**inputs)
    if budgets not in _cache:
        _cache[budgets] = _build(list(budgets))
    nc = _cache[budgets]
    res = run_bass_kernel_spmd(
        nc, in_maps, core_ids=list(range(N_CORES)), trace=trace
    )
    outs = [res.results[k]["out"] for k in range(N_CORES)]
    full = np.concatenate([o[:, :APC] for o in outs], axis=1).T
    full = np.ascontiguousarray(full, dtype=np.float32)
    delta_q = full[:, 0:1]
    f_out = full[:, 1:2]
    delta_a = full[:, 2:]
    return (delta_a, delta_q, f_out), res


def kernel(**inputs):
    out, _ = _run(inputs, trace=False)
    return out
